# revision 2
# baseline (speedup 1.0000x reference)
"""Trainium2 Bass kernel v2 for nn_CorrelationFilter (SiamFC-style correlation).

Math (per batch pair b):
    out[b, oi, oj] = sum_{di<6, dj<6, c<256} x[b, oi+di, oj+dj, c] * z[b, di, dj, c]
                     + sum_{c<256} bias[0, oi, oj, b*256 + c]

Strategy: data parallel over batch across 8 cores (16 batches/core). Per batch,
ONE matmul per channel-half streams x [128c, 484] against a stationary z
[128c, 44] whose 7-spaced columns hold all 36 filter taps (row 1+7di+dj) plus a
zero col 0 reserved for the bias row; two fp8 ones-matmuls accumulate the bias
channel-sum into PSUM row 0. The 44-row PSUM block is evacuated (bf16) to a
DRAM scratch, re-gathered with per-tap column shifts (22di+dj) applied via
affine DRAM addressing (one DMA per (bb,di) + a dense->22-wide bias-row
expansion), and a final ones-masked matmul reduces all 36 tap rows + bias row
of a batch pair into out16[2p:2p+2] - accumulated across pairs into a single
[16, 374] PSUM tile. A strided copy extracts the dense 17x17 output.
"""

import os
import numpy as np
import ml_dtypes

import concourse.bass as bass
import concourse.mybir as mybir
from concourse import bacc
from concourse.tile import TileContext

B, H, W, C = 128, 22, 22, 256
HZ, WZ = 6, 6
HO, WO = 17, 17
OO = HO * WO               # 289 dense output positions
NCORES = 8
BPC = B // NCORES          # 16 batches per core
P = H * W                  # 484 flattened search positions
O22 = (HO - 1) * W + WO    # 369: output span in 22-wide layout
NT = HO * W                # 374: t-tile cols (divisible 17*22 for bias view)
M44 = 1 + 7 * HZ + 1       # 44 stationary cols: bias row 0 + 7-spaced taps
NPAIR = BPC // 2           # 8 psum pairs
SROW = 64 + M44            # 108 scr rows per pair

_BF16 = mybir.dt.bfloat16
_F32 = mybir.dt.float32
_FP8 = mybir.dt.float8e4


def build_module():
    nc = bacc.Bacc()
    xt_d = nc.dram_tensor("xt", [128, 2, BPC, P], _BF16, kind="ExternalInput")
    zt_d = nc.dram_tensor("zt", [128, 2, BPC, M44], _BF16, kind="ExternalInput")
    bt_d = nc.dram_tensor("bt", [128, BPC, 2, NT], _FP8, kind="ExternalInput")
    s1_d = nc.dram_tensor("s1", [128, NPAIR, BPC], _BF16, kind="ExternalInput")
    out_d = nc.dram_tensor("out", [BPC, HO, WO], _F32, kind="ExternalOutput")

    with TileContext(nc) as tc:
        with (
            tc.tile_pool(name="const", bufs=1) as cpool,
            tc.tile_pool(name="big", bufs=1) as big,
            tc.tile_pool(name="work", bufs=2) as work,
            tc.tile_pool(name="psum", bufs=7, space="PSUM") as psum,
            tc.tile_pool(name="psout", bufs=1, space="PSUM") as psout,
            tc.tile_pool(name="dram", bufs=1, space="DRAM") as dpool,
        ):
            # Engine->stage assignment (engine streams execute IN ORDER, so no
            # engine may mix early-stage and late-stage work across pairs):
            #   sync:   input loads only (zt, xt, bt) + final out. Never waits.
            #   tensor: Q matmuls; reduce(p) scheduled 2 pairs late.
            #   vector: evac casts + scr write dispatch (own natural chain).
            #   scalar: constants + dedicated gather engine (scr completes in
            #           pair order, so in-order gathers pipeline cleanly).
            ones8 = cpool.tile([128, 1], _FP8, name="ones8")
            nc.gpsimd.memset(ones8[:], 1.0)
            s1 = cpool.tile([128, NPAIR, BPC], _BF16, name="s1")
            nc.scalar.dma_start(out=s1[:], in_=s1_d[:])
            zt = cpool.tile([128, 2, BPC, M44], _BF16, name="zt")
            nc.scalar.dma_start(out=zt[:], in_=zt_d[:])

            xt = big.tile([128, 2, BPC, P], _BF16, name="xt")
            bt = big.tile([128, BPC, 2, NT], _FP8, name="bt")
            # first pair's x + bias immediately; rest in few big contiguous DMAs
            nc.sync.dma_start(out=xt[:, :, 0:2, :], in_=xt_d[:, :, 0:2, :])
            nc.scalar.dma_start(out=bt[:, 0:2, :, :], in_=bt_d[:, 0:2, :, :])
            XCH = {0: (2, 6), 2: (6, 10), 4: (10, 16)}

            # one T tile spanning all pairs: tap rows on partitions, (pair, m)
            # in the free dim, so one gather DMA covers 4 pairs at once.
            # Zeroed once: unwritten rows/cols stay finite-zero and are masked
            # by s1 in the reduce matmul (bias rows/junk cols NEED the zero).
            T = big.tile([128, NPAIR, NT], _BF16, name="T")
            nc.gpsimd.memset(T[:], 0.0)

            scr = dpool.tile([NPAIR, SROW, P], _BF16, name="scr")
            out16 = psout.tile([BPC, NT], _F32, name="out16")

            def emit_gathers(p0, nphase):
                # 2 bias + 12 tap gathers per phase, split across both rings
                base = scr[:].offset + p0 * SROW * P
                for bb in range(2):
                    eng = nc.sync if bb == 0 else nc.scalar
                    rb = 64 * bb
                    dstb = T[rb : rb + 1, p0 : p0 + nphase, :]
                    srcb = bass.AP(
                        scr[:].tensor, base + rb * P,
                        [[SROW * P, nphase], [1, NT]],
                    )
                    eng.dma_start(out=dstb, in_=srcb)
                    for di in range(HZ):
                        dstf = T[rb + 1 + 7 * di : rb + 7 + 7 * di,
                                 p0 : p0 + nphase, 0:O22]
                        srcf = bass.AP(
                            scr[:].tensor,
                            base + (rb + 1 + 7 * di) * P + 22 * di,
                            [[P + 1, WZ], [SROW * P, nphase], [1, O22]],
                        )
                        eng.dma_start(out=dstf, in_=srcf)

            def emit_reduce(p):
                nc.tensor.matmul(
                    out16[:], s1[:, p, :], T[:, p, 0:NT],
                    start=(p == 0), stop=(p == NPAIR - 1),
                )

            PH = NPAIR // 2
            for p in range(NPAIR):
                if p == 1:   # bulk of the bias stream, off the head critical path
                    nc.scalar.dma_start(
                        out=bt[:, 2:BPC, :, :], in_=bt_d[:, 2:BPC, :, :]
                    )
                if p in XCH:  # stream remaining search features during compute
                    a, bnd = XCH[p]
                    nc.sync.dma_start(
                        out=xt[:, :, a:bnd, :], in_=xt_d[:, :, a:bnd, :]
                    )
                evb = work.tile([128, P], _BF16, name="evb", tag="evb", bufs=4)
                for e in range(2):
                    b = 2 * p + e
                    q = psum.tile([128, P], _F32, name="q", tag="q")
                    nc.tensor.matmul(
                        q[0:M44, :], zt[:, 0, b, :], xt[:, 0, b, :],
                        start=True, stop=False,
                    )
                    for ch in range(2):
                        nc.tensor.matmul(
                            q[0:1, 0:NT], ones8[:], bt[:, b, ch, :],
                            start=False, stop=False,
                        )
                    nc.tensor.matmul(
                        q[0:M44, :], zt[:, 1, b, :], xt[:, 1, b, :],
                        start=False, stop=True,
                    )
                    # evacuate + downcast PSUM f32 -> SBUF bf16 in one pass
                    nc.vector.tensor_copy(
                        out=evb[64 * e : 64 * e + M44, :], in_=q[0:M44, :]
                    )
                nc.scalar.dma_start(out=scr[p, :, :], in_=evb[0:SROW, :])
                if p == PH:          # pairs 0..PH-1 bounced; gather them now
                    emit_gathers(0, PH)
                if p == NPAIR - 2:   # phase-A reduces overlap last pairs
                    for pr in range(PH):
                        emit_reduce(pr)
            emit_gathers(PH, NPAIR - PH)
            for pr in range(PH, NPAIR):
                emit_reduce(pr)

            outb = work.tile([BPC, HO, WO], _F32, name="outb")
            o16v = out16[:].rearrange("b (i j) -> b i j", j=W)[:, :, 0:WO]
            nc.vector.tensor_copy(out=outb[:], in_=o16v)
            nc.sync.dma_start(out=out_d[:], in_=outb[:])

    nc.compile()
    return nc


def prep_inputs(x, z, b):
    """Host-side shard + layout prep. Returns per-core in_maps."""
    xb = np.asarray(x).astype(ml_dtypes.bfloat16)
    zb = np.asarray(z).astype(ml_dtypes.bfloat16)
    bf = np.clip(np.asarray(b), -240.0, 240.0).astype(ml_dtypes.float8_e4m3)
    bias3 = bf.reshape(OO, B, C)

    # s1 mask + fp8 ones column (shared by all cores)
    s1 = np.zeros((128, NPAIR, BPC), dtype=ml_dtypes.bfloat16)
    for bb in range(2):
        for di in range(HZ):
            for dj in range(WZ):
                r = 64 * bb + 1 + 7 * di + dj
                for p in range(NPAIR):
                    s1[r, p, 2 * p + bb] = 1.0
    for bb in range(2):  # bias rows
        for p in range(NPAIR):
            s1[64 * bb, p, 2 * p + bb] = 1.0
    in_maps = []
    for core in range(NCORES):
        b0 = core * BPC
        xs = xb[b0 : b0 + BPC].reshape(BPC, P, C)
        # xT[c, ch, b, p] with c fastest on partitions: channel ch*128 + c
        xT = np.ascontiguousarray(
            xs.reshape(BPC, P, 2, 128).transpose(3, 2, 0, 1)
        )
        zs = zb[b0 : b0 + BPC]                      # [BPC, 6, 6, C]
        zT = np.zeros((128, 2, BPC, M44), dtype=ml_dtypes.bfloat16)
        zr = zs.reshape(BPC, HZ, WZ, 2, 128).transpose(4, 3, 0, 1, 2)
        for di in range(HZ):
            zT[:, :, :, 1 + 7 * di : 7 + 7 * di] = zr[:, :, :, di, :]
        bs = bias3[:, b0 : b0 + BPC, :]             # [OO, BPC, C]
        bTd = bs.reshape(OO, BPC, 2, 128).transpose(3, 1, 2, 0)
        bT = np.zeros((128, BPC, 2, NT), dtype=ml_dtypes.float8_e4m3)
        o22 = (np.arange(OO) // WO) * W + (np.arange(OO) % WO)
        bT[:, :, :, o22] = bTd
        in_maps.append({"xt": xT, "zt": zT, "bt": bT, "s1": s1})
    return in_maps


_cache = {}


def _ensure_ntff_hook():
    try:
        from antenv.axon_hooks import get_axon_ntff_profile_hook  # noqa: F401
        return True
    except ImportError:
        pass
    try:
        import sys, types
        from trn_agent_boot.trn_boot import _ntff_profile_via_ctypes

        so = os.environ.get("AXON_PJRT_SO", "/opt/axon/libaxon_pjrt.so")
        hook = _ntff_profile_via_ctypes(so)
        mod = types.ModuleType("antenv.axon_hooks")
        mod.get_axon_ntff_profile_hook = lambda: hook
        mod.set_axon_ntff_profile_hook = lambda h: None
        sys.modules["antenv.axon_hooks"] = mod
        import antenv

        antenv.axon_hooks = mod
        return True
    except Exception:
        return False


def kernel(x, z, b):
    from concourse.bass_utils import run_bass_kernel_spmd

    if "nc" not in _cache:
        _cache["nc"] = build_module()
    nc = _cache["nc"]
    in_maps = prep_inputs(x, z, b)
    trace = bool(int(os.environ.get("KERNEL_TRACE", "0") or 0))
    if trace:
        trace = _ensure_ntff_hook()
    res = run_bass_kernel_spmd(
        nc,
        in_maps,
        core_ids=list(range(NCORES)),
        trace=trace,
    )
    _cache["last_result"] = res
    out = np.concatenate([r["out"].reshape(BPC, HO, WO) for r in res.results], axis=0)
    return out[..., None].astype(np.float32)
